# revision 8
# baseline (speedup 1.0000x reference)
"""Trainium2 Bass kernel for nn_CombinedBandPassFilter.

Computes y[b, 0, f, t] = sum_k x[b, 0, t+k-384] * kernels[f, k]  (conv1d,
'same' padding, K=769, 40 filters, B=32, T=32768).

Strategy (8 NeuronCores, filter-sharded, 5 filters per core):
  Block-Toeplitz matmul formulation. Output chunked t = 256*a + s with
  a in [0,128) as the PSUM partition dim and s in [0,256) as the free dim:

      y[256a + s] = sum_j sum_r x[256a + 128j + OFF + r] * W_j[r, s]
      W_j[r, s]   = h[384 + 128j + OFF + r - s]

  The stationary matmul operand is a [128, 128] stride-2-column slice of x
  stored block-column-major in SBUF (Xmat[r, c] = x[128c + SHIFT + r]); the
  moving operand is the precomputed filter-Toeplitz block W_j [128, 256]
  (float32r: full-rate fp32 at free dim >= 256). PSUM accumulates over j.
  Per-filter offset OFF in {0, -64} (two x layouts in SBUF) minimizes the
  number of j-blocks to ceil((2m + 256)/128) for true tap half-width m,
  exploiting the fact that the 40 padded filters have supports 19..769.

  Filters are assigned to 5 per-core slots with compile-time step counts
  J = [8, 5, 3, 3, 3]; per-core differences live entirely in input data
  (the W blocks), so one SPMD program serves all 8 cores.
"""

import math
import numpy as np

B = 32
T = 32768
KLEN = 769
PADK = 384
N = 256          # output chunk size = psum free dim
A = T // N       # 128 chunks -> psum partition dim
NCORE = 8
NSLOT = 5
NF = 40

_COMPILED = None     # (nc, plan) cache
LAST_RESULT = None   # BassKernelResults from the most recent run (for test.py)


# ---------------------------------------------------------------- filters ---
def _design_filter(fs, low_hz, high_hz, cycle):
    n_taps = int(cycle * fs / low_hz)
    if n_taps % 2 == 0:
        n_taps += 1
    m = (n_taps - 1) / 2.0
    k = np.arange(n_taps) - m
    fl, fh = low_hz / fs, high_hz / fs
    h = 2.0 * fh * np.sinc(2.0 * fh * k) - 2.0 * fl * np.sinc(2.0 * fl * k)
    w = 0.54 - 0.46 * np.cos(2.0 * np.pi * np.arange(n_taps) / (n_taps - 1))
    h = h * w
    fc = 0.5 * (low_hz + high_hz) / fs
    resp = np.abs(np.sum(h * np.exp(-2j * np.pi * fc * k)))
    return (h / resp).astype(np.float32)


def _build_kernels():
    FS, CYCLE_PHA, CYCLE_AMP = 512, 3, 6
    pha = [(l, l + 2) for l in range(2, 22)]
    amp = [(l, l + 20) for l in range(60, 160, 5)]
    filters = [_design_filter(FS, l, h, CYCLE_PHA) for (l, h) in pha]
    filters += [_design_filter(FS, l, h, CYCLE_AMP) for (l, h) in amp]
    max_len = max(f.shape[0] for f in filters)
    padded = []
    for f in filters:
        pad = max_len - f.shape[0]
        padded.append(np.pad(f, (pad // 2, pad - pad // 2)))
    return np.stack(padded).astype(np.float32)


# ------------------------------------------------------------------- plan ---
class Plan:
    pass


def _make_plan(kernels):
    """Assign filters to (core, slot); compute per-slot (fam, jmin, J)."""
    per_f = []
    for f in range(NF):
        nz = np.nonzero(kernels[f])[0]
        m = int(max(PADK - nz[0], nz[-1] - PADK)) if len(nz) else 0
        best = None
        for fam, OFF in ((0, 0), (1, -64)):
            jmin = math.floor((-m - OFF) / 128)
            jmax = math.floor((255 + m - OFF) / 128)
            nj = jmax - jmin + 1
            if best is None or nj < best[4]:
                best = (fam, OFF, jmin, jmax, nj)
        per_f.append(best)

    famA = [f for f in range(NF) if per_f[f][0] == 0]
    famB = [f for f in range(NF) if per_f[f][0] == 1]
    assert len(famA) == NCORE and len(famB) == NCORE * (NSLOT - 1), (
        f"unexpected family split {len(famA)}/{len(famB)}")

    famA.sort(key=lambda f: -per_f[f][4])
    famB.sort(key=lambda f: -per_f[f][4])

    # assign[c][k] = filter index
    assign = [[None] * NSLOT for _ in range(NCORE)]
    slot_fam = [0] + [1] * (NSLOT - 1)
    slot_jmin = []
    slot_J = []
    for k in range(NSLOT):
        group = famA if k == 0 else famB[(k - 1) * NCORE:k * NCORE]
        for c in range(NCORE):
            assign[c][k] = group[c]
        jmin = min(per_f[f][2] for f in group)
        jmax = max(per_f[f][3] for f in group)
        slot_jmin.append(jmin)
        slot_J.append(jmax - jmin + 1)

    p = Plan()
    p.per_f = per_f
    p.assign = assign
    p.slot_fam = slot_fam
    p.slot_jmin = slot_jmin
    p.slot_J = slot_J
    p.SJ = sum(slot_J)
    # x layout column ranges per family: cols c = 2a + j, a in [0,A), with
    # x-sample index 128*c + OFF + r.  col range [jmin_fam, 254 + jmax_fam].
    p.fam_off = [0, -64]
    p.fam_cmin = []
    p.fam_C = []
    for fam in range(2):
        ks = [k for k in range(NSLOT) if slot_fam[k] == fam]
        jmin = min(slot_jmin[k] for k in ks)
        jmax = max(slot_jmin[k] + slot_J[k] - 1 for k in ks)
        p.fam_cmin.append(jmin)
        p.fam_C.append(254 + jmax - jmin + 1)
    return p


# -------------------------------------------------------------- host prep ---
def _prep_x(x, plan):
    """Build the two block-column-major x layouts, [128, B*C] each."""
    xf = np.ascontiguousarray(x.reshape(B, T), dtype=np.float32)
    LPAD = 1024
    xp = np.zeros((B, LPAD + T + LPAD), np.float32)
    xp[:, LPAD:LPAD + T] = xf
    outs = []
    for fam in range(2):
        C = plan.fam_C[fam]
        cmin = plan.fam_cmin[fam]
        OFF = plan.fam_off[fam]
        # col c (0-based) holds x[128*(c + cmin) + OFF + r]
        start = LPAD + 128 * cmin + OFF
        v = np.lib.stride_tricks.as_strided(
            xp[:, start:], shape=(B, C, 128),
            strides=(xp.strides[1] * xp.shape[1], 512, 4))
        outs.append(np.ascontiguousarray(v.transpose(2, 0, 1)).reshape(128, B * C))
    return outs


def _build_W(h, OFF, j):
    r = np.arange(128)[:, None]
    s = np.arange(N)[None, :]
    k = PADK + 128 * j + OFF + r - s
    valid = (k >= 0) & (k < KLEN)
    W = np.zeros((128, N), np.float32)
    W[valid] = h[np.clip(k, 0, KLEN - 1)][valid]
    return W


def _prep_w(kernels, plan):
    """Per-core moving-operand blocks, [128, SJ*N]."""
    ws = []
    for c in range(NCORE):
        Wc = np.zeros((plan.SJ, 128, N), np.float32)
        idx = 0
        for k in range(NSLOT):
            f = plan.assign[c][k]
            OFF = plan.fam_off[plan.slot_fam[k]]
            for t in range(plan.slot_J[k]):
                j = plan.slot_jmin[k] + t
                if plan.per_f[f][2] <= j <= plan.per_f[f][3]:
                    Wc[idx] = _build_W(kernels[f], OFF, j)
                idx += 1
        ws.append(np.ascontiguousarray(Wc.transpose(1, 0, 2)).reshape(128, plan.SJ * N))
    return ws


# ---------------------------------------------------------------- program ---
def _build_program(plan):
    import concourse.bacc as bacc
    import concourse.mybir as mybir
    from concourse.tile import TileContext

    F32R = mybir.dt.float32r
    F32 = mybir.dt.float32

    nc = bacc.Bacc("TRN2", target_bir_lowering=False)
    # One blob input = [xa | xb | w]: a single DMA means every consumer
    # waits on one semaphore (the fused fp32r LDW slot allows only one).
    xa_cols = B * plan.fam_C[0]
    xb_cols = B * plan.fam_C[1]
    w_cols = plan.SJ * N
    CT = xa_cols + xb_cols + w_cols
    x_base = [0, xa_cols]
    w_base = xa_cols + xb_cols
    blob_d = nc.dram_tensor("blob", [128, CT], F32R, kind="ExternalInput")
    y_d = nc.dram_tensor("y", [B, NSLOT, T], F32, kind="ExternalOutput")
    y_ap = y_d.ap()

    cum = [0] * NSLOT
    acc = 0
    for k in range(NSLOT):
        cum[k] = acc
        acc += plan.slot_J[k]

    with TileContext(nc) as tc:
        with (
            tc.tile_pool(name="const", bufs=1) as cpool,
            tc.tile_pool(name="psum", bufs=8, space="PSUM") as ppool,
            tc.tile_pool(name="ev", bufs=6) as epool,
        ):
            blob = cpool.tile([128, CT], F32R)
            nc.sync.dma_start(blob[:], blob_d[:])

            for b in range(B):
                for k in range(NSLOT):
                    fam = plan.slot_fam[k]
                    C = plan.fam_C[fam]
                    cmin = plan.fam_cmin[fam]
                    ps = ppool.tile([128, N], F32, tag="ps")
                    J = plan.slot_J[k]
                    for t in range(J):
                        j = plan.slot_jmin[k] + t
                        col0 = x_base[fam] + b * C + (j - cmin)
                        lhsT = blob[:, col0:col0 + 256:2]
                        rhs = blob[:, w_base + (cum[k] + t) * N:
                                   w_base + (cum[k] + t + 1) * N]
                        nc.tensor.matmul(ps[:], lhsT, rhs,
                                         start=(t == 0), stop=(t == J - 1))
                    ev = epool.tile([128, N], F32, tag="ev")
                    nc.vector.tensor_copy(ev[:], ps[:])
                    yv = y_ap[b, k].rearrange("(a n) -> a n", n=N)
                    nc.sync.dma_start(yv, ev[:])
    nc.finalize()
    return nc


# ----------------------------------------------------------------- kernel ---
def kernel(x, kernels=None):
    global _COMPILED, LAST_RESULT
    import os
    from concourse.bass_utils import run_bass_kernel_spmd

    x = np.asarray(x, dtype=np.float32)
    if kernels is None:
        kernels = _build_kernels()
    kernels = np.asarray(kernels, dtype=np.float32)
    assert x.shape == (B, 1, T) and kernels.shape == (NF, KLEN)

    if _COMPILED is None:
        plan = _make_plan(kernels)
        nc = _build_program(plan)
        _COMPILED = (nc, plan)
    nc, plan = _COMPILED

    xa, xb = _prep_x(x, plan)
    ws = _prep_w(kernels, plan)
    in_maps = [{"blob": np.concatenate([xa, xb, ws[c]], axis=1)}
               for c in range(NCORE)]

    trace = bool(int(os.environ.get("KERNEL_TRACE", "0")))
    try:
        res = run_bass_kernel_spmd(nc, in_maps, core_ids=list(range(NCORE)),
                                   trace=trace)
    except Exception:
        if not trace:
            raise
        res = run_bass_kernel_spmd(nc, in_maps, core_ids=list(range(NCORE)),
                                   trace=False)
    LAST_RESULT = res

    out = np.empty((B, 1, NF, T), np.float32)
    for c in range(NCORE):
        yc = res.results[c]["y"]  # [B, NSLOT, T]
        for k in range(NSLOT):
            out[:, 0, plan.assign[c][k], :] = yc[:, k, :]
    return out


# revision 11
# speedup vs baseline: 1.0210x; 1.0210x over previous
"""Trainium2 Bass kernel for nn_CombinedBandPassFilter.

Computes y[b, 0, f, t] = sum_k x[b, 0, t+k-384] * kernels[f, k]  (conv1d,
'same' padding, K=769, 40 filters, B=32, T=32768).

Strategy (8 NeuronCores, filter-sharded, 5 filters per core):
  Block-Toeplitz matmul formulation. Output chunked t = 256*a + s with
  a in [0,128) as the PSUM partition dim and s in [0,256) as the free dim:

      y[256a + s] = sum_j sum_r x[256a + 128j + OFF + r] * W_j[r, s]
      W_j[r, s]   = h[384 + 128j + OFF + r - s]

  The stationary matmul operand is a [128, 128] stride-2-column slice of x
  stored block-column-major in SBUF (Xmat[r, c] = x[128c + SHIFT + r]); the
  moving operand is the precomputed filter-Toeplitz block W_j. PSUM
  accumulates over j. Per-filter offset OFF in {0, -64} (two x layouts)
  minimizes the j-block count to ceil((2m + 256)/128) for true tap
  half-width m, exploiting the wildly varying filter supports (19..769).

  Filters are assigned to 5 per-core slots with compile-time step counts
  [8, 5, 3, 3, 3]; per-core differences live entirely in input data (the
  W blocks), so one SPMD program serves all 8 cores.  The two identical
  3-step slots are fused into one N=512 matmul stream (shared stationary).
  x is loaded in 8 batch-group tiles so input DMA overlaps compute.
"""

import math
import os
import numpy as np

B = 32
T = 32768
KLEN = 769
PADK = 384
N = 256          # output chunk size = psum free dim
A = T // N       # 128 chunks -> psum partition dim
NCORE = 8
NSLOT = 5
NF = 40
GB = 4           # batches per x-group tile
NG = B // GB

_COMPILED = {}
LAST_RESULT = None   # BassKernelResults of the most recent run (for test.py)


def _dtype_cfg():
    """(mybir dtype, numpy dtype) for the matmul operands."""
    import concourse.mybir as mybir
    kind = os.environ.get("KERNEL_DTYPE", "f32r")
    if kind == "f32r":
        return kind, mybir.dt.float32r, np.float32
    if kind == "f16":
        return kind, mybir.dt.float16, np.float16
    if kind == "bf16":
        import ml_dtypes
        return kind, mybir.dt.bfloat16, ml_dtypes.bfloat16
    raise ValueError(kind)


# ---------------------------------------------------------------- filters ---
def _design_filter(fs, low_hz, high_hz, cycle):
    n_taps = int(cycle * fs / low_hz)
    if n_taps % 2 == 0:
        n_taps += 1
    m = (n_taps - 1) / 2.0
    k = np.arange(n_taps) - m
    fl, fh = low_hz / fs, high_hz / fs
    h = 2.0 * fh * np.sinc(2.0 * fh * k) - 2.0 * fl * np.sinc(2.0 * fl * k)
    w = 0.54 - 0.46 * np.cos(2.0 * np.pi * np.arange(n_taps) / (n_taps - 1))
    h = h * w
    fc = 0.5 * (low_hz + high_hz) / fs
    resp = np.abs(np.sum(h * np.exp(-2j * np.pi * fc * k)))
    return (h / resp).astype(np.float32)


def _build_kernels():
    FS, CYCLE_PHA, CYCLE_AMP = 512, 3, 6
    pha = [(l, l + 2) for l in range(2, 22)]
    amp = [(l, l + 20) for l in range(60, 160, 5)]
    filters = [_design_filter(FS, l, h, CYCLE_PHA) for (l, h) in pha]
    filters += [_design_filter(FS, l, h, CYCLE_AMP) for (l, h) in amp]
    max_len = max(f.shape[0] for f in filters)
    padded = []
    for f in filters:
        pad = max_len - f.shape[0]
        padded.append(np.pad(f, (pad // 2, pad - pad // 2)))
    return np.stack(padded).astype(np.float32)


# ------------------------------------------------------------------- plan ---
class Plan:
    pass


def _make_plan(kernels):
    """Assign filters to (core, slot); compute per-slot (fam, jmin, J)."""
    per_f = []
    for f in range(NF):
        nz = np.nonzero(kernels[f])[0]
        m = int(max(PADK - nz[0], nz[-1] - PADK)) if len(nz) else 0
        best = None
        for fam, OFF in ((0, 0), (1, -64)):
            jmin = math.floor((-m - OFF) / 128)
            jmax = math.floor((255 + m - OFF) / 128)
            nj = jmax - jmin + 1
            if best is None or nj < best[4]:
                best = (fam, OFF, jmin, jmax, nj)
        per_f.append(best)

    famA = [f for f in range(NF) if per_f[f][0] == 0]
    famB = [f for f in range(NF) if per_f[f][0] == 1]
    assert len(famA) == NCORE and len(famB) == NCORE * (NSLOT - 1), (
        f"unexpected family split {len(famA)}/{len(famB)}")

    famA.sort(key=lambda f: -per_f[f][4])
    famB.sort(key=lambda f: -per_f[f][4])

    assign = [[None] * NSLOT for _ in range(NCORE)]
    slot_fam = [0] + [1] * (NSLOT - 1)
    slot_jmin = []
    slot_J = []
    for k in range(NSLOT):
        group = famA if k == 0 else famB[(k - 1) * NCORE:k * NCORE]
        for c in range(NCORE):
            assign[c][k] = group[c]
        jmin = min(per_f[f][2] for f in group)
        jmax = max(per_f[f][3] for f in group)
        slot_jmin.append(jmin)
        slot_J.append(jmax - jmin + 1)

    p = Plan()
    p.per_f = per_f
    p.assign = assign
    p.slot_fam = slot_fam
    p.slot_jmin = slot_jmin
    p.slot_J = slot_J

    # Fuse slots with identical (fam, jmin, J) pairwise into N=512 streams.
    groups = []   # list of (slots_tuple, fam, jmin, J, width)
    used = [False] * NSLOT
    for k in range(NSLOT):
        if used[k]:
            continue
        partner = None
        for k2 in range(k + 1, NSLOT):
            if (not used[k2]
                    and slot_fam[k2] == slot_fam[k]
                    and slot_jmin[k2] == slot_jmin[k]
                    and slot_J[k2] == slot_J[k]):
                partner = k2
                break
        if partner is None:
            groups.append(((k,), slot_fam[k], slot_jmin[k], slot_J[k], N))
            used[k] = True
        else:
            groups.append(((k, partner), slot_fam[k], slot_jmin[k],
                           slot_J[k], 2 * N))
            used[k] = used[partner] = True
    p.groups = groups
    # W layout: per group, per step, a [128, width] block. cumulative cols:
    p.w_steps = []   # (group_idx, step, col_offset, width)
    col = 0
    for gi, (slots, fam, jmin, J, width) in enumerate(groups):
        for t in range(J):
            p.w_steps.append((gi, t, col, width))
            col += width
    p.w_cols = col

    # x layout column ranges per family: cols c = 2a + j with sample index
    # 128*(c + cmin) + OFF + r.
    p.fam_off = [0, -64]
    p.fam_cmin = []
    p.fam_C = []
    for fam in range(2):
        ks = [k for k in range(NSLOT) if slot_fam[k] == fam]
        jmin = min(slot_jmin[k] for k in ks)
        jmax = max(slot_jmin[k] + slot_J[k] - 1 for k in ks)
        p.fam_cmin.append(jmin)
        p.fam_C.append(254 + jmax - jmin + 1)
    p.xb_cols = p.fam_C[0] + p.fam_C[1]        # per-batch cols (famA|famB)
    return p


# -------------------------------------------------------------- host prep ---
def _prep_x(x, plan, npdt):
    """Block-column-major x, per-batch [famA | famB]: [128, B*xb_cols]."""
    xf = np.ascontiguousarray(x.reshape(B, T), dtype=np.float32)
    LPAD = 1024
    xp = np.zeros((B, LPAD + T + LPAD), np.float32)
    xp[:, LPAD:LPAD + T] = xf
    fams = []
    for fam in range(2):
        C = plan.fam_C[fam]
        start = LPAD + 128 * plan.fam_cmin[fam] + plan.fam_off[fam]
        v = np.lib.stride_tricks.as_strided(
            xp[:, start:], shape=(B, C, 128),
            strides=(xp.strides[0], 512, 4))
        fams.append(v)
    out = np.concatenate(fams, axis=1)            # [B, xb_cols, 128]
    out = np.ascontiguousarray(out.transpose(2, 0, 1), dtype=npdt)
    return out.reshape(128, B * plan.xb_cols)


def _build_W(h, OFF, j):
    r = np.arange(128)[:, None]
    s = np.arange(N)[None, :]
    k = PADK + 128 * j + OFF + r - s
    valid = (k >= 0) & (k < KLEN)
    W = np.zeros((128, N), np.float32)
    W[valid] = h[np.clip(k, 0, KLEN - 1)][valid]
    return W


def _prep_w(kernels, plan, npdt):
    """Per-core moving-operand blocks: [128, w_cols]."""
    ws = []
    for c in range(NCORE):
        Wc = np.zeros((128, plan.w_cols), np.float32)
        for (gi, t, col, width) in plan.w_steps:
            slots, fam, jmin, J, _ = plan.groups[gi]
            OFF = plan.fam_off[fam]
            j = jmin + t
            for si, k in enumerate(slots):
                f = plan.assign[c][k]
                if plan.per_f[f][2] <= j <= plan.per_f[f][3]:
                    Wc[:, col + si * N:col + (si + 1) * N] = \
                        _build_W(kernels[f], OFF, j)
        ws.append(np.ascontiguousarray(Wc.astype(npdt)))
    return ws


# ---------------------------------------------------------------- program ---
def _build_program(plan, mmdt):
    import concourse.bacc as bacc
    import concourse.mybir as mybir
    from concourse.tile import TileContext

    F32 = mybir.dt.float32

    nc = bacc.Bacc("TRN2", target_bir_lowering=False)
    x_d = nc.dram_tensor("x", [128, B * plan.xb_cols], mmdt,
                         kind="ExternalInput")
    w_d = nc.dram_tensor("w", [128, plan.w_cols], mmdt, kind="ExternalInput")
    y_d = nc.dram_tensor("y", [B, NSLOT, T], F32, kind="ExternalOutput")
    y_ap = y_d.ap()

    GCOLS = GB * plan.xb_cols

    with TileContext(nc) as tc:
        with (
            tc.tile_pool(name="wconst", bufs=1) as wpool,
            tc.tile_pool(name="xg", bufs=NG) as xgpool,
            tc.tile_pool(name="psum", bufs=8, space="PSUM") as ppool,
            tc.tile_pool(name="ev", bufs=6) as epool,
        ):
            w_s = wpool.tile([128, plan.w_cols], mmdt)
            nc.sync.dma_start(w_s[:], w_d[:])
            xg_tiles = []
            for g in range(NG):
                xt = xgpool.tile([128, GCOLS], mmdt, tag="xg")
                nc.sync.dma_start(xt[:], x_d[:, g * GCOLS:(g + 1) * GCOLS])
                xg_tiles.append(xt)

            for b in range(B):
                xt = xg_tiles[b // GB]
                bcol = (b % GB) * plan.xb_cols
                for gi, (slots, fam, jmin, J, width) in enumerate(plan.groups):
                    fbase = bcol + (plan.fam_C[0] if fam == 1 else 0)
                    cmin = plan.fam_cmin[fam]
                    ps = ppool.tile([128, width], F32, tag="ps")
                    for t in range(J):
                        j = jmin + t
                        col0 = fbase + (j - cmin)
                        lhsT = xt[:, col0:col0 + 255:2]
                        _, _, wcol, _ = plan.w_steps[
                            [i for i, st in enumerate(plan.w_steps)
                             if st[0] == gi and st[1] == t][0]]
                        rhs = w_s[:, wcol:wcol + width]
                        nc.tensor.matmul(ps[:], lhsT, rhs,
                                         start=(t == 0), stop=(t == J - 1))
                    ev = epool.tile([128, width], F32, tag="ev")
                    nc.vector.tensor_copy(ev[:], ps[:])
                    for si, k in enumerate(slots):
                        yv = y_ap[b, k].rearrange("(a n) -> a n", n=N)
                        nc.sync.dma_start(yv, ev[:, si * N:(si + 1) * N])
    nc.finalize()
    return nc


# ----------------------------------------------------------------- kernel ---
def kernel(x, kernels=None):
    global LAST_RESULT
    from concourse.bass_utils import run_bass_kernel_spmd

    x = np.asarray(x, dtype=np.float32)
    if kernels is None:
        kernels = _build_kernels()
    kernels = np.asarray(kernels, dtype=np.float32)
    assert x.shape == (B, 1, T) and kernels.shape == (NF, KLEN)

    kind, mmdt, npdt = _dtype_cfg()
    if kind not in _COMPILED:
        plan = _make_plan(kernels)
        nc = _build_program(plan, mmdt)
        _COMPILED[kind] = (nc, plan)
    nc, plan = _COMPILED[kind]

    xh = _prep_x(x, plan, npdt)
    ws = _prep_w(kernels, plan, npdt)
    in_maps = [{"x": xh, "w": ws[c]} for c in range(NCORE)]

    trace = bool(int(os.environ.get("KERNEL_TRACE", "0")))
    try:
        res = run_bass_kernel_spmd(nc, in_maps, core_ids=list(range(NCORE)),
                                   trace=trace)
    except Exception:
        if not trace:
            raise
        res = run_bass_kernel_spmd(nc, in_maps, core_ids=list(range(NCORE)),
                                   trace=False)
    LAST_RESULT = res

    out = np.empty((B, 1, NF, T), np.float32)
    for c in range(NCORE):
        yc = res.results[c]["y"]  # [B, NSLOT, T]
        for k in range(NSLOT):
            out[:, 0, plan.assign[c][k], :] = yc[:, k, :]
    return out


# revision 13
# speedup vs baseline: 1.1029x; 1.0803x over previous
"""Trainium2 Bass kernel for nn_CombinedBandPassFilter.

Computes y[b, 0, f, t] = sum_k x[b, 0, t+k-384] * kernels[f, k]  (conv1d,
'same' padding, K=769, 40 filters, B=32, T=32768).

Strategy (8 NeuronCores, filter-sharded, 5 filters per core):
  Block-Toeplitz matmul formulation. Output chunked t = 256*a + s with
  a in [0,128) as the PSUM partition dim and s in [0,256) as the free dim:

      y[256a + s] = sum_j sum_r x[256a + 128j + OFF + r] * W_j[r, s]
      W_j[r, s]   = h[384 + 128j + OFF + r - s]

  The stationary matmul operand is a [128, 128] stride-2-column slice of x
  stored block-column-major in SBUF (Xmat[r, c] = x[128c + SHIFT + r]); the
  moving operand is the precomputed filter-Toeplitz block W_j. PSUM
  accumulates over j. Per-filter offset OFF in {0, -64} (two x layouts)
  minimizes the j-block count to ceil((2m + 256)/128) for true tap
  half-width m, exploiting the wildly varying filter supports (19..769).

  Filters are assigned to 5 per-core slots with compile-time step counts
  [8, 5, 3, 3, 3]; per-core differences live entirely in input data (the
  W blocks), so one SPMD program serves all 8 cores.  The two identical
  3-step slots are fused into one N=512 matmul stream (shared stationary).
  x is loaded in 8 batch-group tiles so input DMA overlaps compute.
"""

import math
import os
import numpy as np

B = 32
T = 32768
KLEN = 769
PADK = 384
N = 256          # output chunk size = psum free dim
A = T // N       # 128 chunks -> psum partition dim
NCORE = 8
NSLOT = 5
NF = 40
GB = 4           # batches per x-group tile
NG = B // GB

_COMPILED = {}
LAST_RESULT = None   # BassKernelResults of the most recent run (for test.py)


def _dtype_cfg():
    """(mybir dtype, numpy dtype) for the matmul operands."""
    import concourse.mybir as mybir
    kind = os.environ.get("KERNEL_DTYPE", "f32r")
    if kind == "f32r":
        return kind, mybir.dt.float32r, np.float32
    if kind == "f16":
        return kind, mybir.dt.float16, np.float16
    if kind == "bf16":
        import ml_dtypes
        return kind, mybir.dt.bfloat16, ml_dtypes.bfloat16
    raise ValueError(kind)


# ---------------------------------------------------------------- filters ---
def _design_filter(fs, low_hz, high_hz, cycle):
    n_taps = int(cycle * fs / low_hz)
    if n_taps % 2 == 0:
        n_taps += 1
    m = (n_taps - 1) / 2.0
    k = np.arange(n_taps) - m
    fl, fh = low_hz / fs, high_hz / fs
    h = 2.0 * fh * np.sinc(2.0 * fh * k) - 2.0 * fl * np.sinc(2.0 * fl * k)
    w = 0.54 - 0.46 * np.cos(2.0 * np.pi * np.arange(n_taps) / (n_taps - 1))
    h = h * w
    fc = 0.5 * (low_hz + high_hz) / fs
    resp = np.abs(np.sum(h * np.exp(-2j * np.pi * fc * k)))
    return (h / resp).astype(np.float32)


def _build_kernels():
    FS, CYCLE_PHA, CYCLE_AMP = 512, 3, 6
    pha = [(l, l + 2) for l in range(2, 22)]
    amp = [(l, l + 20) for l in range(60, 160, 5)]
    filters = [_design_filter(FS, l, h, CYCLE_PHA) for (l, h) in pha]
    filters += [_design_filter(FS, l, h, CYCLE_AMP) for (l, h) in amp]
    max_len = max(f.shape[0] for f in filters)
    padded = []
    for f in filters:
        pad = max_len - f.shape[0]
        padded.append(np.pad(f, (pad // 2, pad - pad // 2)))
    return np.stack(padded).astype(np.float32)


# ------------------------------------------------------------------- plan ---
class Plan:
    pass


def _make_plan(kernels):
    """Assign filters to (core, slot); compute per-slot (fam, jmin, J)."""
    per_f = []
    for f in range(NF):
        nz = np.nonzero(kernels[f])[0]
        m = int(max(PADK - nz[0], nz[-1] - PADK)) if len(nz) else 0
        best = None
        for fam, OFF in ((0, 0), (1, -64)):
            jmin = math.floor((-m - OFF) / 128)
            jmax = math.floor((255 + m - OFF) / 128)
            nj = jmax - jmin + 1
            if best is None or nj < best[4]:
                best = (fam, OFF, jmin, jmax, nj)
        per_f.append(best)

    famA = [f for f in range(NF) if per_f[f][0] == 0]
    famB = [f for f in range(NF) if per_f[f][0] == 1]
    assert len(famA) == NCORE and len(famB) == NCORE * (NSLOT - 1), (
        f"unexpected family split {len(famA)}/{len(famB)}")

    famA.sort(key=lambda f: -per_f[f][4])
    famB.sort(key=lambda f: -per_f[f][4])

    assign = [[None] * NSLOT for _ in range(NCORE)]
    slot_fam = [0] + [1] * (NSLOT - 1)
    slot_jmin = []
    slot_J = []
    for k in range(NSLOT):
        group = famA if k == 0 else famB[(k - 1) * NCORE:k * NCORE]
        for c in range(NCORE):
            assign[c][k] = group[c]
        jmin = min(per_f[f][2] for f in group)
        jmax = max(per_f[f][3] for f in group)
        slot_jmin.append(jmin)
        slot_J.append(jmax - jmin + 1)

    p = Plan()
    p.per_f = per_f
    p.assign = assign
    p.slot_fam = slot_fam
    p.slot_jmin = slot_jmin
    p.slot_J = slot_J

    # Fuse slots with identical (fam, jmin, J) pairwise into N=512 streams.
    groups = []   # list of (slots_tuple, fam, jmin, J, width)
    used = [False] * NSLOT
    for k in range(NSLOT):
        if used[k]:
            continue
        partner = None
        for k2 in range(k + 1, NSLOT):
            if (not used[k2]
                    and slot_fam[k2] == slot_fam[k]
                    and slot_jmin[k2] == slot_jmin[k]
                    and slot_J[k2] == slot_J[k]):
                partner = k2
                break
        if partner is None:
            groups.append(((k,), slot_fam[k], slot_jmin[k], slot_J[k], N))
            used[k] = True
        else:
            groups.append(((k, partner), slot_fam[k], slot_jmin[k],
                           slot_J[k], 2 * N))
            used[k] = used[partner] = True
    p.groups = groups
    # W layout: per group, per step, a [128, width] block. cumulative cols:
    p.w_steps = []   # (group_idx, step, col_offset, width)
    col = 0
    for gi, (slots, fam, jmin, J, width) in enumerate(groups):
        for t in range(J):
            p.w_steps.append((gi, t, col, width))
            col += width
    p.w_cols = col

    # x layout column ranges per family: cols c = 2a + j with sample index
    # 128*(c + cmin) + OFF + r.
    p.fam_off = [0, -64]
    p.fam_cmin = []
    p.fam_C = []
    for fam in range(2):
        ks = [k for k in range(NSLOT) if slot_fam[k] == fam]
        jmin = min(slot_jmin[k] for k in ks)
        jmax = max(slot_jmin[k] + slot_J[k] - 1 for k in ks)
        p.fam_cmin.append(jmin)
        p.fam_C.append(254 + jmax - jmin + 1)
    p.xb_cols = p.fam_C[0] + p.fam_C[1]        # per-batch cols (famA|famB)
    return p


# -------------------------------------------------------------- host prep ---
def _prep_x(x, plan, npdt):
    """Block-column-major x, per-batch [famA | famB]: [128, B*xb_cols]."""
    xf = np.ascontiguousarray(x.reshape(B, T), dtype=np.float32)
    LPAD = 1024
    xp = np.zeros((B, LPAD + T + LPAD), np.float32)
    xp[:, LPAD:LPAD + T] = xf
    fams = []
    for fam in range(2):
        C = plan.fam_C[fam]
        start = LPAD + 128 * plan.fam_cmin[fam] + plan.fam_off[fam]
        v = np.lib.stride_tricks.as_strided(
            xp[:, start:], shape=(B, C, 128),
            strides=(xp.strides[0], 512, 4))
        fams.append(v)
    out = np.concatenate(fams, axis=1)            # [B, xb_cols, 128]
    out = np.ascontiguousarray(out.transpose(2, 0, 1), dtype=npdt)
    return out.reshape(128, B * plan.xb_cols)


def _build_W(h, OFF, j):
    r = np.arange(128)[:, None]
    s = np.arange(N)[None, :]
    k = PADK + 128 * j + OFF + r - s
    valid = (k >= 0) & (k < KLEN)
    W = np.zeros((128, N), np.float32)
    W[valid] = h[np.clip(k, 0, KLEN - 1)][valid]
    return W


def _prep_w(kernels, plan, npdt):
    """Per-core moving-operand blocks: [128, w_cols]."""
    ws = []
    for c in range(NCORE):
        Wc = np.zeros((128, plan.w_cols), np.float32)
        for (gi, t, col, width) in plan.w_steps:
            slots, fam, jmin, J, _ = plan.groups[gi]
            OFF = plan.fam_off[fam]
            j = jmin + t
            for si, k in enumerate(slots):
                f = plan.assign[c][k]
                if plan.per_f[f][2] <= j <= plan.per_f[f][3]:
                    Wc[:, col + si * N:col + (si + 1) * N] = \
                        _build_W(kernels[f], OFF, j)
        ws.append(np.ascontiguousarray(Wc.astype(npdt)))
    return ws


# ---------------------------------------------------------------- program ---
def _build_program(plan, mmdt):
    import concourse.bacc as bacc
    import concourse.mybir as mybir
    from concourse.tile import TileContext

    F32 = mybir.dt.float32

    nc = bacc.Bacc("TRN2", target_bir_lowering=False)
    x_d = nc.dram_tensor("x", [128, B * plan.xb_cols], mmdt,
                         kind="ExternalInput")
    w_d = nc.dram_tensor("w", [128, plan.w_cols], mmdt, kind="ExternalInput")
    y_d = nc.dram_tensor("y", [B, NSLOT, T], F32, kind="ExternalOutput")
    y_ap = y_d.ap()

    GCOLS = GB * plan.xb_cols

    with TileContext(nc) as tc:
        with (
            tc.tile_pool(name="wconst", bufs=1) as wpool,
            tc.tile_pool(name="xg", bufs=NG) as xgpool,
            tc.tile_pool(name="psum", bufs=8, space="PSUM") as ppool,
            tc.tile_pool(name="ev", bufs=6) as epool,
        ):
            # Three DMA paths run in parallel: W on the gpsimd SWDGE queue,
            # x groups on the SP HWDGE queue, outputs on the ACT HWDGE queue.
            w_s = wpool.tile([128, plan.w_cols], mmdt)
            nc.gpsimd.dma_start(w_s[:], w_d[:])
            xg_tiles = []
            for g in range(NG):
                xt = xgpool.tile([128, GCOLS], mmdt, tag="xg")
                nc.sync.dma_start(xt[:], x_d[:, g * GCOLS:(g + 1) * GCOLS])
                xg_tiles.append(xt)

            for b in range(B):
                xt = xg_tiles[b // GB]
                bcol = (b % GB) * plan.xb_cols
                for gi, (slots, fam, jmin, J, width) in enumerate(plan.groups):
                    fbase = bcol + (plan.fam_C[0] if fam == 1 else 0)
                    cmin = plan.fam_cmin[fam]
                    ps = ppool.tile([128, width], F32, tag="ps")
                    for t in range(J):
                        j = jmin + t
                        col0 = fbase + (j - cmin)
                        lhsT = xt[:, col0:col0 + 255:2]
                        _, _, wcol, _ = plan.w_steps[
                            [i for i, st in enumerate(plan.w_steps)
                             if st[0] == gi and st[1] == t][0]]
                        rhs = w_s[:, wcol:wcol + width]
                        nc.tensor.matmul(ps[:], lhsT, rhs,
                                         start=(t == 0), stop=(t == J - 1))
                    ev = epool.tile([128, width], F32, tag="ev")
                    nc.vector.tensor_copy(ev[:], ps[:])
                    for si, k in enumerate(slots):
                        yv = y_ap[b, k].rearrange("(a n) -> a n", n=N)
                        nc.scalar.dma_start(yv, ev[:, si * N:(si + 1) * N])
    nc.finalize()
    return nc


# ----------------------------------------------------------------- kernel ---
def kernel(x, kernels=None):
    global LAST_RESULT
    from concourse.bass_utils import run_bass_kernel_spmd

    x = np.asarray(x, dtype=np.float32)
    if kernels is None:
        kernels = _build_kernels()
    kernels = np.asarray(kernels, dtype=np.float32)
    assert x.shape == (B, 1, T) and kernels.shape == (NF, KLEN)

    kind, mmdt, npdt = _dtype_cfg()
    if kind not in _COMPILED:
        plan = _make_plan(kernels)
        nc = _build_program(plan, mmdt)
        _COMPILED[kind] = (nc, plan)
    nc, plan = _COMPILED[kind]

    xh = _prep_x(x, plan, npdt)
    ws = _prep_w(kernels, plan, npdt)
    in_maps = [{"x": xh, "w": ws[c]} for c in range(NCORE)]

    trace = bool(int(os.environ.get("KERNEL_TRACE", "0")))
    try:
        res = run_bass_kernel_spmd(nc, in_maps, core_ids=list(range(NCORE)),
                                   trace=trace)
    except Exception:
        if not trace:
            raise
        res = run_bass_kernel_spmd(nc, in_maps, core_ids=list(range(NCORE)),
                                   trace=False)
    LAST_RESULT = res

    out = np.empty((B, 1, NF, T), np.float32)
    for c in range(NCORE):
        yc = res.results[c]["y"]  # [B, NSLOT, T]
        for k in range(NSLOT):
            out[:, 0, plan.assign[c][k], :] = yc[:, k, :]
    return out


# revision 16
# speedup vs baseline: 1.4591x; 1.3229x over previous
"""Trainium2 Bass kernel for nn_CombinedBandPassFilter.

Computes y[b, 0, f, t] = sum_k x[b, 0, t+k-384] * kernels[f, k]  (conv1d,
'same' padding, K=769, 40 filters, B=32, T=32768).

Strategy (8 NeuronCores, filter-sharded, 5 filters per core):
  Block-Toeplitz matmul formulation. Output chunked t = 256*a + s with
  a in [0,128) as the PSUM partition dim and s in [0,256) as the free dim:

      y[256a + s] = sum_j sum_r x[256a + 128j + OFF + r] * W_j[r, s]
      W_j[r, s]   = h[384 + 128j + OFF + r - s]

  The stationary matmul operand is a [128, 128] stride-2-column slice of x
  stored block-column-major in SBUF (Xmat[r, c] = x[128c + SHIFT + r]); the
  moving operand is the precomputed filter-Toeplitz block W_j. PSUM
  accumulates over j. Per-filter offset OFF in {0, -64} (two x layouts)
  minimizes the j-block count to ceil((2m + 256)/128) for true tap
  half-width m, exploiting the wildly varying filter supports (19..769).

  Filters are assigned to 5 per-core slots with compile-time step counts
  [8, 5, 3, 3, 3]; per-core differences live entirely in input data (the
  W blocks), so one SPMD program serves all 8 cores.  The two identical
  3-step slots are fused into one N=512 matmul stream (shared stationary).
  x is loaded in 8 batch-group tiles so input DMA overlaps compute.
"""

import math
import os
import numpy as np

B = 32
T = 32768
KLEN = 769
PADK = 384
N = 256          # output chunk size = psum free dim
A = T // N       # 128 chunks -> psum partition dim
NCORE = 8
NSLOT = 5
NF = 40
GB = 4           # batches per x-group tile
NG = B // GB

_COMPILED = {}
LAST_RESULT = None   # BassKernelResults of the most recent run (for test.py)


def _dtype_cfg():
    """(mybir dtype, numpy dtype) for the matmul operands."""
    import concourse.mybir as mybir
    kind = os.environ.get("KERNEL_DTYPE", "f32r")
    if kind == "f32r":
        return kind, mybir.dt.float32r, np.float32
    if kind == "f16":
        return kind, mybir.dt.float16, np.float16
    if kind == "bf16":
        import ml_dtypes
        return kind, mybir.dt.bfloat16, ml_dtypes.bfloat16
    raise ValueError(kind)


# ---------------------------------------------------------------- filters ---
def _design_filter(fs, low_hz, high_hz, cycle):
    n_taps = int(cycle * fs / low_hz)
    if n_taps % 2 == 0:
        n_taps += 1
    m = (n_taps - 1) / 2.0
    k = np.arange(n_taps) - m
    fl, fh = low_hz / fs, high_hz / fs
    h = 2.0 * fh * np.sinc(2.0 * fh * k) - 2.0 * fl * np.sinc(2.0 * fl * k)
    w = 0.54 - 0.46 * np.cos(2.0 * np.pi * np.arange(n_taps) / (n_taps - 1))
    h = h * w
    fc = 0.5 * (low_hz + high_hz) / fs
    resp = np.abs(np.sum(h * np.exp(-2j * np.pi * fc * k)))
    return (h / resp).astype(np.float32)


def _build_kernels():
    FS, CYCLE_PHA, CYCLE_AMP = 512, 3, 6
    pha = [(l, l + 2) for l in range(2, 22)]
    amp = [(l, l + 20) for l in range(60, 160, 5)]
    filters = [_design_filter(FS, l, h, CYCLE_PHA) for (l, h) in pha]
    filters += [_design_filter(FS, l, h, CYCLE_AMP) for (l, h) in amp]
    max_len = max(f.shape[0] for f in filters)
    padded = []
    for f in filters:
        pad = max_len - f.shape[0]
        padded.append(np.pad(f, (pad // 2, pad - pad // 2)))
    return np.stack(padded).astype(np.float32)


# ------------------------------------------------------------------- plan ---
class Plan:
    pass


def _make_plan(kernels):
    """Assign filters to (core, slot); compute per-slot (fam, jmin, J)."""
    per_f = []
    for f in range(NF):
        nz = np.nonzero(kernels[f])[0]
        m = int(max(PADK - nz[0], nz[-1] - PADK)) if len(nz) else 0
        best = None
        for fam, OFF in ((0, 0), (1, -64)):
            jmin = math.floor((-m - OFF) / 128)
            jmax = math.floor((255 + m - OFF) / 128)
            nj = jmax - jmin + 1
            if best is None or nj < best[4]:
                best = (fam, OFF, jmin, jmax, nj)
        per_f.append(best)

    famA = [f for f in range(NF) if per_f[f][0] == 0]
    famB = [f for f in range(NF) if per_f[f][0] == 1]
    assert len(famA) == NCORE and len(famB) == NCORE * (NSLOT - 1), (
        f"unexpected family split {len(famA)}/{len(famB)}")

    famA.sort(key=lambda f: -per_f[f][4])
    famB.sort(key=lambda f: -per_f[f][4])

    assign = [[None] * NSLOT for _ in range(NCORE)]
    slot_fam = [0] + [1] * (NSLOT - 1)
    slot_jmin = []
    slot_J = []
    for k in range(NSLOT):
        group = famA if k == 0 else famB[(k - 1) * NCORE:k * NCORE]
        for c in range(NCORE):
            assign[c][k] = group[c]
        jmin = min(per_f[f][2] for f in group)
        jmax = max(per_f[f][3] for f in group)
        slot_jmin.append(jmin)
        slot_J.append(jmax - jmin + 1)

    p = Plan()
    p.per_f = per_f
    p.assign = assign
    p.slot_fam = slot_fam
    p.slot_jmin = slot_jmin
    p.slot_J = slot_J

    # Fuse slots with identical (fam, jmin, J) pairwise into N=512 streams.
    groups = []   # list of (slots_tuple, fam, jmin, J, width)
    used = [False] * NSLOT
    for k in range(NSLOT):
        if used[k]:
            continue
        partner = None
        for k2 in range(k + 1, NSLOT):
            if (not used[k2]
                    and slot_fam[k2] == slot_fam[k]
                    and slot_jmin[k2] == slot_jmin[k]
                    and slot_J[k2] == slot_J[k]):
                partner = k2
                break
        if partner is None:
            groups.append(((k,), slot_fam[k], slot_jmin[k], slot_J[k], N))
            used[k] = True
        else:
            groups.append(((k, partner), slot_fam[k], slot_jmin[k],
                           slot_J[k], 2 * N))
            used[k] = used[partner] = True
    p.groups = groups
    # W layout: per group, per step, a [128, width] block. cumulative cols:
    p.w_steps = []   # (group_idx, step, col_offset, width)
    col = 0
    for gi, (slots, fam, jmin, J, width) in enumerate(groups):
        for t in range(J):
            p.w_steps.append((gi, t, col, width))
            col += width
    p.w_cols = col

    # x layout column ranges per family: cols c = 2a + j with sample index
    # 128*(c + cmin) + OFF + r.
    p.fam_off = [0, -64]
    p.fam_cmin = []
    p.fam_C = []
    for fam in range(2):
        ks = [k for k in range(NSLOT) if slot_fam[k] == fam]
        jmin = min(slot_jmin[k] for k in ks)
        jmax = max(slot_jmin[k] + slot_J[k] - 1 for k in ks)
        p.fam_cmin.append(jmin)
        p.fam_C.append(254 + jmax - jmin + 1)
    p.xb_cols = p.fam_C[0] + p.fam_C[1]        # per-batch cols (famA|famB)
    return p


# -------------------------------------------------------------- host prep ---
def _prep_x(x, plan, npdt):
    """Block-column-major x, per-batch [famA | famB]: [128, B*xb_cols]."""
    xf = np.ascontiguousarray(x.reshape(B, T), dtype=np.float32)
    LPAD = 1024
    xp = np.zeros((B, LPAD + T + LPAD), np.float32)
    xp[:, LPAD:LPAD + T] = xf
    fams = []
    for fam in range(2):
        C = plan.fam_C[fam]
        start = LPAD + 128 * plan.fam_cmin[fam] + plan.fam_off[fam]
        v = np.lib.stride_tricks.as_strided(
            xp[:, start:], shape=(B, C, 128),
            strides=(xp.strides[0], 512, 4))
        fams.append(v)
    out = np.concatenate(fams, axis=1)            # [B, xb_cols, 128]
    out = np.ascontiguousarray(out.transpose(2, 0, 1), dtype=npdt)
    return out.reshape(128, B * plan.xb_cols)


def _build_W(h, OFF, j):
    r = np.arange(128)[:, None]
    s = np.arange(N)[None, :]
    k = PADK + 128 * j + OFF + r - s
    valid = (k >= 0) & (k < KLEN)
    W = np.zeros((128, N), np.float32)
    W[valid] = h[np.clip(k, 0, KLEN - 1)][valid]
    return W


def _prep_w(kernels, plan, npdt):
    """Per-core moving-operand blocks: [128, w_cols]."""
    ws = []
    for c in range(NCORE):
        Wc = np.zeros((128, plan.w_cols), np.float32)
        for (gi, t, col, width) in plan.w_steps:
            slots, fam, jmin, J, _ = plan.groups[gi]
            OFF = plan.fam_off[fam]
            j = jmin + t
            for si, k in enumerate(slots):
                f = plan.assign[c][k]
                if plan.per_f[f][2] <= j <= plan.per_f[f][3]:
                    Wc[:, col + si * N:col + (si + 1) * N] = \
                        _build_W(kernels[f], OFF, j)
        ws.append(np.ascontiguousarray(Wc.astype(npdt)))
    return ws


# ---------------------------------------------------------------- program ---
def _build_program(plan, mmdt):
    import concourse.bacc as bacc
    import concourse.mybir as mybir
    from concourse.tile import TileContext

    F32 = mybir.dt.float32

    nc = bacc.Bacc("TRN2", target_bir_lowering=False)
    x_d = nc.dram_tensor("x", [128, B * plan.xb_cols], mmdt,
                         kind="ExternalInput")
    w_d = nc.dram_tensor("w", [128, plan.w_cols], mmdt, kind="ExternalInput")
    y_d = nc.dram_tensor("y", [B, NSLOT, T], F32, kind="ExternalOutput")
    y_ap = y_d.ap()

    GCOLS = GB * plan.xb_cols

    with TileContext(nc) as tc:
        _rr = [0]
        with (
            tc.tile_pool(name="wconst", bufs=1) as wpool,
            tc.tile_pool(name="xg", bufs=NG) as xgpool,
            tc.tile_pool(name="psum", bufs=8, space="PSUM") as ppool,
            tc.tile_pool(name="ev", bufs=10) as epool,
        ):
            # Spread traffic over the three DMA paths (SP-HWDGE, ACT-HWDGE,
            # gpsimd-SWDGE): W on gpsimd (group-0 blocks first so compute can
            # start), x groups alternating SP/ACT, outputs round-robin on all
            # three.
            w_s = wpool.tile([128, plan.w_cols], mmdt)
            g0_cols = plan.groups[0][3] * plan.groups[0][4]   # J0 * width0
            nc.gpsimd.dma_start(w_s[:, :g0_cols], w_d[:, :g0_cols])
            nc.gpsimd.dma_start(w_s[:, g0_cols:], w_d[:, g0_cols:])
            xg_tiles = []
            for g in range(NG):
                xt = xgpool.tile([128, GCOLS], mmdt, tag="xg")
                eng = nc.sync if g % 2 == 0 else nc.scalar
                eng.dma_start(xt[:], x_d[:, g * GCOLS:(g + 1) * GCOLS])
                xg_tiles.append(xt)

            for b in range(B):
                xt = xg_tiles[b // GB]
                bcol = (b % GB) * plan.xb_cols
                for gi, (slots, fam, jmin, J, width) in enumerate(plan.groups):
                    fbase = bcol + (plan.fam_C[0] if fam == 1 else 0)
                    cmin = plan.fam_cmin[fam]
                    ps = ppool.tile([128, width], F32, tag="ps")
                    for t in range(J):
                        j = jmin + t
                        col0 = fbase + (j - cmin)
                        lhsT = xt[:, col0:col0 + 255:2]
                        _, _, wcol, _ = plan.w_steps[
                            [i for i, st in enumerate(plan.w_steps)
                             if st[0] == gi and st[1] == t][0]]
                        rhs = w_s[:, wcol:wcol + width]
                        nc.tensor.matmul(ps[:], lhsT, rhs,
                                         start=(t == 0), stop=(t == J - 1))
                    ev = epool.tile([128, width], F32, tag="ev")
                    nc.vector.tensor_copy(ev[:], ps[:])
                    for si, k in enumerate(slots):
                        yv = y_ap[b, k].rearrange("(a n) -> a n", n=N)
                        eng = (nc.sync, nc.scalar, nc.gpsimd)[_rr[0] % 3]
                        _rr[0] += 1
                        eng.dma_start(yv, ev[:, si * N:(si + 1) * N])
    nc.finalize()
    return nc


# ----------------------------------------------------------------- kernel ---
def kernel(x, kernels=None):
    global LAST_RESULT
    from concourse.bass_utils import run_bass_kernel_spmd

    x = np.asarray(x, dtype=np.float32)
    if kernels is None:
        kernels = _build_kernels()
    kernels = np.asarray(kernels, dtype=np.float32)
    assert x.shape == (B, 1, T) and kernels.shape == (NF, KLEN)

    kind, mmdt, npdt = _dtype_cfg()
    if kind not in _COMPILED:
        plan = _make_plan(kernels)
        nc = _build_program(plan, mmdt)
        _COMPILED[kind] = (nc, plan)
    nc, plan = _COMPILED[kind]

    xh = _prep_x(x, plan, npdt)
    ws = _prep_w(kernels, plan, npdt)
    in_maps = [{"x": xh, "w": ws[c]} for c in range(NCORE)]

    trace = bool(int(os.environ.get("KERNEL_TRACE", "0")))
    try:
        res = run_bass_kernel_spmd(nc, in_maps, core_ids=list(range(NCORE)),
                                   trace=trace)
    except Exception:
        if not trace:
            raise
        res = run_bass_kernel_spmd(nc, in_maps, core_ids=list(range(NCORE)),
                                   trace=False)
    LAST_RESULT = res

    out = np.empty((B, 1, NF, T), np.float32)
    for c in range(NCORE):
        yc = res.results[c]["y"]  # [B, NSLOT, T]
        for k in range(NSLOT):
            out[:, 0, plan.assign[c][k], :] = yc[:, k, :]
    return out
